# revision 48
# baseline (speedup 1.0000x reference)
"""Multi-head attention (B=4, S=2048, D=1024, H=16, Hd=64) on 8 trn2 cores.

Sharding: core c = (batch b = c // 2, head-group hg = c % 2). Each core
computes attention for 8 heads of one batch and the corresponding slice of
the output projection; host sums the two partial outputs per batch.

Schedule: the kernel is PE-bound overall while the attention inner loop is
ScalarE(exp)-locked (256 exps of [128,1024] at ~1005 ns are the clock), so
all projection work that is not needed for the first attention chunk is
interleaved into the exp-locked attention phase where the PE otherwise
idles waiting on exp:
  - DMA priority: xt, then Wk + the first Wq slices, Wv, rest.
  - Prologue: all 16 kT groups + qT for the first two chunks.
  - Chunk (p0,c0) runs with the 16 v-projection groups interleaved
    (attnV[i] consumes v[st=i] just after it is produced).
  - Chunks are ordered c-major so each q-range's o-projection becomes
    ready every 4 chunks and spreads evenly as backfill.
  - Explicit scheduler priority bands make the scores+exp stream win every
    ready-race (band 0); attnV, chunk infra and qT projection halves run
    next (band 1, elastic via 8 et buffers); o-projection halves and the
    deferred normalization backfill at band 2. Filler drains use the
    Vector engine so the Scalar engine runs exp back-to-back.
  - Softmax denominators ride as the 65th row of the attnV matmuls;
    normalization is deferred ~one chunk (reciprocal_approx_fast, DVE) and
    applied via K=1 ones-matmul broadcasts + a DVE multiply.

Per-core layout (all matmuls bf16 with fp32 PSUM accumulation):
  xt   = x[b].T                    [D=1024, S=2048]  (lhsT/rhs K-major)
  qT/kT = (Wslice.T @ ..)          [512, 2048]  d-major, 4 pair-tiles of 128
  v    = x @ Wv_slice              [2048, 512]  s-major (+ones col per head)
  per head-pair chunk: scoresT[k,q] (row-tiled K=64 pair) -> exp
            outT[d,q] += v-block.T @ expT ; denom[q] += ones row
  y = outT.T-blocks @ Wo_slice + bo   [2048, 1024] bf16 partial
"""

import numpy as np
import ml_dtypes

S = 2048
D = 1024
HG_D = 512          # head dims per core (8 heads x 64)
NH = 8              # heads per core
KT = S // 128       # 16 k-tiles
DT = D // 128       # 8 contraction tiles for QKV
ST = S // 128       # 16 s-tiles
OT = HG_D // 128    # 4 contraction tiles for O-proj / pair tiles
N_CORES = 8

BF16 = ml_dtypes.bfloat16

_CACHED_NC = {}

# Chunk order (p = head-pair tile, c = 512-wide q chunk): c-major so all
# four p-chunks of a given c finish every 4 chunks and its o-projection
# spreads evenly through the attention phase as filler work. All kT groups
# are computed in the prologue, so chunks never block on filler prereqs;
# only the qT group of each chunk (cheap, one per chunk) rides as filler.
CHUNK_ORDER = [(p, c) for c in range(4) for p in range(4)]


def _build_nc(with_bq=True, with_bk=True, with_bv=True, with_bo=True):
    import concourse.bass as bass  # noqa: F401
    import concourse.mybir as mybir
    import concourse.tile as tile
    from concourse import bacc

    f32 = mybir.dt.float32
    bf16 = mybir.dt.bfloat16
    Exp = mybir.ActivationFunctionType.Exp

    nc = bacc.Bacc("TRN2", target_bir_lowering=False, debug=False,
                   num_devices=N_CORES)

    xt_d = nc.dram_tensor("xt", [D, S], bf16, kind="ExternalInput")
    wq_d = nc.dram_tensor("wq", [D, HG_D], bf16, kind="ExternalInput")
    wk_d = nc.dram_tensor("wk", [D, HG_D], bf16, kind="ExternalInput")
    wv_d = nc.dram_tensor("wv", [D, HG_D], bf16, kind="ExternalInput")
    wo_d = nc.dram_tensor("wo", [HG_D, D], bf16, kind="ExternalInput")
    bq_d = nc.dram_tensor("bqt", [128, OT], f32, kind="ExternalInput")
    bk_d = nc.dram_tensor("bkt", [128, OT], f32, kind="ExternalInput")
    bv_d = nc.dram_tensor("bvr", [1, HG_D], bf16, kind="ExternalInput")
    bo_d = nc.dram_tensor("bor", [1, D], bf16, kind="ExternalInput")
    y_d = nc.dram_tensor("y", [S, D], bf16, kind="ExternalOutput")

    with tile.TileContext(nc) as tc:
        with (
            tc.tile_pool(name="cpool", bufs=1) as cpool,
            tc.tile_pool(name="wpool", bufs=2) as wpool,
            tc.tile_pool(name="pspool", bufs=3, space="PSUM") as pspool,
            tc.tile_pool(name="popool", bufs=2, space="PSUM") as popool,
        ):
            # ---- persistent SBUF tiles ----
            xt_sb = cpool.tile([128, DT, S], bf16, name="xt_sb")
            wq_sb = cpool.tile([128, DT, HG_D], bf16, name="wq_sb")
            wk_sb = cpool.tile([128, DT, HG_D], bf16, name="wk_sb")
            wv_sb = cpool.tile([128, DT, HG_D], bf16, name="wv_sb")
            wo_sb = cpool.tile([128, OT, D], bf16, name="wo_sb")
            bq_sb = cpool.tile([128, OT], f32, name="bq_sb")
            bk_sb = cpool.tile([128, OT], f32, name="bk_sb")
            bvr_sb = cpool.tile([1, HG_D], bf16, name="bvr_sb")
            bor_sb = cpool.tile([1, D], bf16, name="bor_sb")
            ones_t = cpool.tile([128, 128], bf16, name="ones_t")
            qT_sb = cpool.tile([128, OT, S], bf16, name="qT_sb")
            kT_sb = cpool.tile([128, OT, S], bf16, name="kT_sb")
            # v with a trailing ones column per head: attnv lhsT [128, 65]
            # whose 65th output row accumulates the softmax denominator.
            v_sb = cpool.tile([128, ST, NH, 65], bf16, name="v_sb")
            aoT_sb = cpool.tile([128, OT, S], bf16, name="aoT_sb")

            # ---- loads (priority order: xt, then p0 slices of wk/wq,
            # then wv, then the rest) ----
            for k in range(DT):
                nc.sync.dma_start(out=xt_sb[:, k, :],
                                  in_=xt_d[k * 128:(k + 1) * 128, :])
            for k in range(DT):
                nc.sync.dma_start(out=wk_sb[:, k, :],
                                  in_=wk_d[k * 128:(k + 1) * 128, :])
                nc.sync.dma_start(out=wq_sb[:, k, 0:256],
                                  in_=wq_d[k * 128:(k + 1) * 128, 0:256])
            for k in range(DT):
                nc.sync.dma_start(out=wv_sb[:, k, :],
                                  in_=wv_d[k * 128:(k + 1) * 128, :])
            for k in range(DT):
                nc.sync.dma_start(out=wq_sb[:, k, 256:512],
                                  in_=wq_d[k * 128:(k + 1) * 128, 256:512])
            for k in range(OT):
                nc.sync.dma_start(out=wo_sb[:, k, :],
                                  in_=wo_d[k * 128:(k + 1) * 128, :])
            nc.sync.dma_start(out=bq_sb[:], in_=bq_d[:])
            nc.sync.dma_start(out=bk_sb[:], in_=bk_d[:])
            nc.sync.dma_start(out=bvr_sb[:], in_=bv_d[:])
            nc.sync.dma_start(out=bor_sb[:], in_=bo_d[:])
            nc.gpsimd.memset(ones_t[:], 1.0)
            nc.vector.memset(v_sb[:], 1.0)

            # ---- projection group emitters ----
            def emit_qk_group(which, p, jc, on_scalar, half=None):
                if which == "q":
                    w_sb, b_sb, out_sb, wb = wq_sb, bq_sb, qT_sb, with_bq
                else:
                    w_sb, b_sb, out_sb, wb = wk_sb, bk_sb, kT_sb, with_bk
                # half=None: full 512-wide group; half=0/1: 256-wide halves
                # (smaller filler bursts absorb into the scores lookahead).
                lo = jc * 512 + (0 if not half else 256)
                w = 512 if half is None else 256
                pq = pspool.tile([128, w], f32, tag="ps", name="pq")
                for k in range(DT):
                    nc.tensor.matmul(
                        pq[:],
                        w_sb[:, k, p * 128:(p + 1) * 128],
                        xt_sb[:, k, lo:lo + w],
                        start=(k == 0), stop=(k == DT - 1),
                    )
                dst = out_sb[:, p, lo:lo + w]
                if wb:
                    nc.scalar.add(dst, pq[:], b_sb[:, p:p + 1])
                elif on_scalar:
                    nc.scalar.copy(dst, pq[:])
                else:
                    nc.vector.tensor_copy(dst, pq[:])

            def emit_v_group(st, half=None):
                lo = 0 if not half else 256
                w = 512 if half is None else 256
                pv = pspool.tile([128, w], f32, tag="ps", name="pv")
                for k in range(DT):
                    nc.tensor.matmul(
                        pv[:],
                        xt_sb[:, k, st * 128:(st + 1) * 128],
                        wv_sb[:, k, lo:lo + w],
                        start=(k == 0), stop=(not with_bv and k == DT - 1),
                    )
                if with_bv:
                    nc.tensor.matmul(pv[:], ones_t[0:1, 0:128],
                                     bvr_sb[0:1, lo:lo + w],
                                     start=False, stop=True)
                h0 = 0 if half is None else 4 * half
                nc.vector.tensor_copy(
                    v_sb[:, st, h0:h0 + w // 64, 0:64],
                    pv.rearrange("p (h c) -> p h c", c=64))

            def emit_oproj_half(st, l, on_scalar=False):
                yt = wpool.tile([128, 512], bf16, tag="y", bufs=3, name="yt")
                py = pspool.tile([128, 512], f32, tag="ps", name="py")
                for kt in range(OT):
                    nc.tensor.matmul(
                        py[:],
                        aoT_sb[:, kt, st * 128:(st + 1) * 128],
                        wo_sb[:, kt, l * 512:(l + 1) * 512],
                        start=(kt == 0),
                        stop=(not with_bo and kt == OT - 1),
                    )
                if with_bo:
                    nc.tensor.matmul(py[:], ones_t[0:1, 0:128],
                                     bor_sb[0:1, l * 512:(l + 1) * 512],
                                     start=False, stop=True)
                if on_scalar:
                    nc.scalar.copy(yt[:], py[:])
                else:
                    nc.vector.tensor_copy(yt[:], py[:])
                nc.sync.dma_start(
                    out=y_d[st * 128:(st + 1) * 128, l * 512:(l + 1) * 512],
                    in_=yt[:])

            def emit_oproj_group(st):
                for l in range(2):
                    emit_oproj_half(st, l)

            # ---- deferred normalization ----
            # pendings arrive in (head A, head B) pairs for one (p, c);
            # flushing broadcasts both reciprocals into one psum tile via
            # K=1 ones-matmuls and scales the 128-partition aoT block with
            # a single DVE multiply. Deferred by one chunk so the PE never
            # waits on the reciprocal.
            import contextlib

            # Scheduling priority bands (Tile scheduler: lower number wins
            # among READY instructions). Band 0: scores+exp — the exp
            # cadence is the kernel clock, its producers must win every
            # ready-drain race. Band 1: attnV + chunk infra (has et-buffer
            # slack). Band 2: qk prereq fillers. Band 3: o-proj backfill.
            band_counters = [0, 10_000_000, 20_000_000, 30_000_000]

            @contextlib.contextmanager
            def band(n):
                saved = tc.cur_priority
                tc.cur_priority = band_counters[n]
                try:
                    yield
                finally:
                    band_counters[n] = tc.cur_priority
                    tc.cur_priority = saved

            pending = []
            # Two filler queues: prereq_fillers (kT/qT producers of upcoming
            # chunks — NORMAL priority, so the scheduler runs them promptly
            # and the next chunk's scores never block on a deprioritized
            # producer) and oproj_fillers (no attention-side consumers —
            # LOW priority band, pure backfill into PE idle).
            prereq_fillers = []
            oproj_fillers = []
            norm_done = {c: 0 for c in range(4)}
            oproj_emitted = set()

            def flush_normalize():
                with band(2):
                    _flush_normalize()

            def _flush_normalize():
                off2, p2, c2, rb2, row = pending.pop(0)
                off3, p3, c3, rb3, row3 = pending.pop(0)
                assert p2 == p3 and c2 == c3
                jb = slice(c2 * 512, (c2 + 1) * 512)
                bt = pspool.tile([128, 512], f32, tag="ps", name="bt")
                nc.tensor.matmul(
                    bt[off2:off2 + 64, :],
                    ones_t[row:row + 1, 0:64],
                    rb2[row:row + 1, :],
                    start=True, stop=True,
                )
                nc.tensor.matmul(
                    bt[off3:off3 + 64, :],
                    ones_t[row3:row3 + 1, 0:64],
                    rb3[row3:row3 + 1, :],
                    start=True, stop=True,
                )
                nc.vector.tensor_mul(
                    aoT_sb[:, p2, jb],
                    aoT_sb[:, p2, jb],
                    bt[:])
                norm_done[c2] += 1
                if norm_done[c2] == OT:
                    for st in range(4 * c2, 4 * c2 + 4):
                        if st not in oproj_emitted:
                            oproj_emitted.add(st)
                            for l in range(2):
                                oproj_fillers.append((st, l))

            def run_prereq_filler():
                item = prereq_fillers.pop(0)
                with band(1):
                    if item[0] == "v":
                        emit_v_group(item[1], half=item[2])
                    else:
                        emit_qk_group(item[0], item[1], item[2],
                                      on_scalar=False, half=item[3])

            def ensure_prereqs(p, c):
                # Safety net: force-emit any not-yet-emitted producers of
                # chunk (p, c) right before the chunk reads them.
                need = [k for k in prereq_fillers if
                        (k[0] == "v" and p >= 2) or
                        (k[0] != "v" and k[1] == p and
                         (k[0] == "k" or k[2] == c))]
                for key in need:
                    prereq_fillers.remove(key)
                    with band(2):
                        if key[0] == "v":
                            emit_v_group(key[1], half=key[2])
                        else:
                            emit_qk_group(key[0], key[1], key[2],
                                          on_scalar=False, half=key[3])

            def run_oproj_filler():
                st, l = oproj_fillers.pop(0)
                with band(2):
                    emit_oproj_half(st, l)

            # ---- attention chunk ----
            def emit_chunk(p, c, v_interleave=False, filler_slots=(),
                           dense_prereq=False):
                ensure_prereqs(p, c)
                jb = slice(c * 512, (c + 1) * 512)
                otA = popool.tile([65, 512], f32, tag="po", name="otA")
                otB = popool.tile([65, 512], f32, tag="po", name="otB")
                for i in range(KT):
                    if v_interleave:
                        with band(1):
                            emit_v_group(i)
                    # combined scores psum: head A in cols 0-511 (PE rows
                    # 0-63), head B in cols 512-1023 (rows 64-127) --
                    # consecutive matmuls use disjoint PE row halves so the
                    # array overlaps them.
                    with band(0):
                        stt = pspool.tile([128, 1024], f32, tag="ps",
                                          name="stt")
                        for off in (0, 64):
                            nc.tensor.matmul(
                                stt[:, off * 8:off * 8 + 512],
                                kT_sb[off:off + 64, p, i * 128:(i + 1) * 128],
                                qT_sb[off:off + 64, p, jb],
                                start=True, stop=True,
                            )
                        et = wpool.tile([128, 1024], bf16, tag="exp", bufs=8,
                                        name="et")
                        nc.scalar.activation(et[:], stt[:], Exp, scale=0.125)
                    with band(1):
                        for ot, hh in ((otA, 0), (otB, 1)):
                            nc.tensor.matmul(
                                ot[:],
                                v_sb[:, i, 2 * p + hh, :],
                                et[:, hh * 512:(hh + 1) * 512],
                                start=(i == 0), stop=(i == KT - 1),
                            )
                    if i == 8:
                        while len(pending) > 2:
                            flush_normalize()
                    if filler_slots:
                        if i in (3, 12) and prereq_fillers:
                            run_prereq_filler()
                        elif i in (9, 10) and oproj_fillers:
                            run_oproj_filler()
                # Drain attn rows (cross-partition for the odd head) first
                # so the PSUM tiles free fast, then gather the denominator
                # rows to partitions 64 (A) / 32 (B) and batch-reciprocal.
                ctx = band(1)
                ctx.__enter__()
                nc.vector.tensor_copy(aoT_sb[0:64, p, jb], otA[0:64, :])
                nc.vector.tensor_copy(aoT_sb[64:128, p, jb], otB[0:64, :])
                den = wpool.tile([33, 512], f32, tag="den", bufs=2,
                                 name="den")
                nc.vector.tensor_copy(den[0:1, :], otA[64:65, :])
                nc.vector.tensor_copy(den[32:33, :], otB[64:65, :])
                rf = wpool.tile([33, 512], f32, tag="rf", name="rf")
                rb = wpool.tile([65, 512], bf16, tag="rb", bufs=3,
                                name="rb")
                # approx_fast needs a partition-0-based AP; rows 1..31 hold
                # stale values (harmless, unread). ~5x faster than
                # reciprocal(), ~3e-6 rel err.
                nc.vector.reciprocal_approx_fast(rf[0:33, :], den[0:33, :])
                nc.vector.tensor_copy(rb[64:65, :], rf[0:1, :])
                nc.vector.tensor_copy(rb[32:33, :], rf[32:33, :])
                pending.append((0, p, c, rb, 64))
                pending.append((64, p, c, rb, 32))
                ctx.__exit__(None, None, None)

            # ---- prereq filler order: derived from chunk-order prereqs ----
            # Prologue covers kT p0/p1 + qT of the first two chunks; the
            # rest (kT p2/p3, one qT per later chunk) is paced as normal-
            # priority fillers in deadline order.
            kq_queued = set()

            def queue_qk(which, p, jc):
                if (which, p, jc) not in kq_queued:
                    kq_queued.add((which, p, jc))
                    prereq_fillers.append((which, p, jc, 0))
                    prereq_fillers.append((which, p, jc, 1))

            for p in range(4):
                for jc in range(4):
                    kq_queued.add(("k", p, jc))
            kq_queued.add(("q", *CHUNK_ORDER[0]))
            kq_queued.add(("q", *CHUNK_ORDER[1]))
            for (p, c) in CHUNK_ORDER[2:]:
                queue_qk("q", p, c)

            # ---- prologue (ScalarE is idle before the first exp) ----
            for p in range(4):
                for jc in range(4):
                    emit_qk_group("k", p, jc, on_scalar=True)
            emit_qk_group("q", *CHUNK_ORDER[0], on_scalar=True)
            emit_qk_group("q", *CHUNK_ORDER[1], on_scalar=True)

            # ---- main schedule ----
            for idx, (p, c) in enumerate(CHUNK_ORDER):
                emit_chunk(p, c, v_interleave=(idx == 0),
                           filler_slots=(() if idx == 0 else (1,)),
                           dense_prereq=(1 <= idx <= 3))

            # ---- tail: flush the last normalizations + remaining o-proj ----
            while prereq_fillers:
                run_prereq_filler()
            while pending:
                flush_normalize()
            n = 0
            while oproj_fillers:
                st, l = oproj_fillers.pop(0)
                emit_oproj_half(st, l, on_scalar=(n % 2 == 1))
                n += 1
            for st in range(ST):
                if st not in oproj_emitted:
                    for l in range(2):
                        emit_oproj_half(st, l, on_scalar=(n % 2 == 1))
                        n += 1

    nc.compile()
    return nc


def get_nc(with_bq=True, with_bk=True, with_bv=True, with_bo=True):
    key = (with_bq, with_bk, with_bv, with_bo)
    if key not in _CACHED_NC:
        _CACHED_NC[key] = _build_nc(*key)
    return _CACHED_NC[key]


def make_in_maps(x, Wq, bq, Wk, bk, Wv, bv, Wo, bo):
    x = np.asarray(x, dtype=np.float32)
    in_maps = []
    for c in range(N_CORES):
        b, hg = c // 2, c % 2
        sl = slice(hg * HG_D, (hg + 1) * HG_D)
        in_maps.append({
            "xt": np.ascontiguousarray(np.asarray(x[b]).T).astype(BF16),
            "wq": np.ascontiguousarray(np.asarray(Wq)[:, sl]).astype(BF16),
            "wk": np.ascontiguousarray(np.asarray(Wk)[:, sl]).astype(BF16),
            "wv": np.ascontiguousarray(np.asarray(Wv)[:, sl]).astype(BF16),
            "wo": np.ascontiguousarray(np.asarray(Wo)[sl, :]).astype(BF16),
            "bqt": np.ascontiguousarray(
                np.asarray(bq, np.float32)[sl].reshape(OT, 128).T),
            "bkt": np.ascontiguousarray(
                np.asarray(bk, np.float32)[sl].reshape(OT, 128).T),
            "bvr": np.asarray(bv, np.float32)[sl].reshape(1, HG_D).astype(BF16),
            "bor": (np.asarray(bo, np.float32) if hg == 0
                    else np.zeros(D, np.float32)).reshape(1, D).astype(BF16),
        })
    return in_maps


def run_cores(in_maps, trace=False, with_bq=True, with_bk=True,
              with_bv=True, with_bo=True):
    try:
        import ntff_shim
        ntff_shim.install()
    except Exception:
        pass
    from concourse.bass_utils import run_bass_kernel_spmd

    nc = get_nc(with_bq, with_bk, with_bv, with_bo)
    return run_bass_kernel_spmd(nc, in_maps, list(range(N_CORES)), trace=trace)


def combine(results):
    y = np.empty((4, S, D), np.float32)
    for b in range(4):
        y[b] = (results[2 * b]["y"].astype(np.float32)
                + results[2 * b + 1]["y"].astype(np.float32))
    return y


def kernel(x, Wq, bq, Wk, bk, Wv, bv, Wo, bo):
    in_maps = make_in_maps(x, Wq, bq, Wk, bk, Wv, bv, Wo, bo)
    flags = dict(
        with_bq=bool(np.any(np.asarray(bq))),
        with_bk=bool(np.any(np.asarray(bk))),
        with_bv=bool(np.any(np.asarray(bv))),
        with_bo=bool(np.any(np.asarray(bo))),
    )
    res = run_cores(in_maps, trace=False, **flags)
    return combine(res.results)
